# revision 12
# baseline (speedup 1.0000x reference)
"""Trainium2 Bass kernel for ComplexCoherency loss.

Reference computation (per full input [B=16, C=2, H=512, W=512], k=5):
    cross_r = sum_c(pr*tr + pi*ti)          [B,H,W]
    cross_i = sum_c(pi*tr - pr*ti)
    abs2_p  = sum_c(pr^2 + pi^2)
    abs2_t  = sum_c(tr^2 + ti^2)
    F_m     = 5x5 VALID box-sum of each map  -> [B,508,508]
    abs_c   = sqrt(num_r^2+num_i^2) / sqrt(den_p*den_t)
    out     = 1 - mean(abs_c)

Sharding: pure data parallel, B=16 split 2-per-core across 8 NeuronCores.
Each core returns per-partition partial sums of abs_c; host reduces.

Per-core pipeline:
  DVE : cross products (TT mult), W-direction box via cumsum scan + shifted
        subtract, finals (squares/add/sub)
  ACT : squares of inputs, ln/exp for sqrt(A/B), exp carries accum_out so the
        final reduction is fused
  PE  : H-direction box-sum fused with the channel/product sum as banded 0/1
        matmuls (float32r = full-rate fp32) accumulating in PSUM
"""

import numpy as np
from contextlib import ExitStack

import concourse.bass as bass
import concourse.bacc as bacc
import concourse.tile as tile
import concourse.mybir as mybir
from concourse.bass_utils import run_bass_kernel_spmd

B, C, H, W = 16, 2, 512, 512
KF = 5                      # filter size (hardcoded)
NCORES = 8
BLOC = B // NCORES          # 2 batches per core
HP = H - KF + 1             # 508
WP = W - KF + 1             # 508
NBLK = H // 128             # 4 partition blocks per image
FD = NBLK * W               # 2048 free-dim elems per image tile
GW = NBLK * WP              # 2032 free-dim elems per box-filtered tile
FULL = 3 * WP               # cols of G holding blocks 0..2 (full 128 rows)
TAIL_ROWS = HP - 3 * 128    # 124 valid rows in block 3

AF = mybir.ActivationFunctionType
ALU = mybir.AluOpType
F32 = mybir.dt.float32
F32R = mybir.dt.float32r
BF16 = mybir.dt.bfloat16


def _make_bands() -> np.ndarray:
    """[4,128,128] fp32: +main, +boundary, -main, -boundary H-box bands.

    main[k, m] = 1 if 0 <= k - m <= 4   (input row k contributes to out row m)
    bnd[k, m]  = 1 if m >= 124 + k, for k in 0..3 (rows of the next block)
    """
    k = np.arange(128)[:, None]
    m = np.arange(128)[None, :]
    main = ((k - m >= 0) & (k - m <= KF - 1)).astype(np.float32)
    bnd = np.zeros((128, 128), np.float32)
    for kk in range(KF - 1):
        bnd[kk, 124 + kk :] = 1.0
    return np.stack([main, bnd, -main, -bnd])


def _build_nc() -> bacc.Bacc:
    nc = bacc.Bacc("TRN2", target_bir_lowering=False, debug=False,
                   num_devices=NCORES)
    ins = {
        name: nc.dram_tensor(name, [BLOC, C, H, W], F32,
                             kind="ExternalInput").ap()
        for name in ("pred_real", "pred_imag", "tgt_real", "tgt_imag")
    }
    bands = nc.dram_tensor("bands", [4, 128, 128], F32,
                           kind="ExternalInput").ap()
    out = nc.dram_tensor("partials", [128, 2 * BLOC], F32,
                         kind="ExternalOutput").ap()
    with tile.TileContext(nc) as tc, ExitStack() as ctx:
        _kernel(ctx, tc, out, ins, bands)
    nc.compile()
    return nc


def _kernel(ctx, tc, out_ap, ins, bands_dram):
    nc = tc.nc
    pool = ctx.enter_context(tc.tile_pool(name="main", bufs=2))
    psum_pool = ctx.enter_context(tc.tile_pool(name="psum", space="PSUM", bufs=2))

    # ---- constants -------------------------------------------------------
    bands_sb = pool.tile([128, 4, 128], F32R, tag="bands", bufs=1)
    nc.sync.dma_start(out=bands_sb,
                      in_=bands_dram.rearrange("i k m -> k i m").bitcast(F32R))
    band_main = (bands_sb[:, 0, :], bands_sb[:, 2, :])   # (+, -) [128,128]
    band_bnd = (bands_sb[0:4, 1, :], bands_sb[0:4, 3, :])  # (+, -) [4,128]

    zeros = pool.tile([128, FD], F32, tag="zeros", bufs=1)
    nc.gpsimd.memset(zeros, 0.0)
    acc = pool.tile([128, 2 * BLOC], F32, tag="acc", bufs=1)
    nc.gpsimd.memset(acc, 0.0)

    GB = {}  # batch -> {map_name: G tile (bf16 box-filtered field)}
    pending = []

    def _flush_wbox(item):
        pb, pname, psum = item
        # ---- W-box: cumsum scan + shifted subtract (DVE) ----
        cs = pool.tile([128, FD + 1], F32, tag="cs", bufs=1)
        nc.gpsimd.memset(cs[:, 0:1], 0.0)
        nc.vector.tensor_tensor_scan(
            out=cs[:, 1:FD + 1], data0=psum, data1=zeros,
            initial=0.0, op0=ALU.add, op1=ALU.add)
        g = pool.tile([128, GW], BF16, tag="g", bufs=8)
        # g[t, w'] = cs[512t + w' + 5] - cs[512t + w']  (cs[0] == 0)
        nc.vector.tensor_sub(
            g.rearrange("p (t w) -> p t w", t=NBLK),
            cs[:, 1:FD + 1].rearrange("p (t w) -> p t w", t=NBLK)
            [:, :, KF - 1:W],
            cs[:, 0:FD].rearrange("p (t w) -> p t w", t=NBLK)[:, :, 0:WP])
        GB[pb][pname] = g

    for b in range(BLOC):
        # ---- load the 8 input images of this batch -----------------------
        img = {}
        for nm, key in (("pr", "pred_real"), ("pi", "pred_imag"),
                        ("tr", "tgt_real"), ("ti", "tgt_imag")):
            for c in range(C):
                t = pool.tile([128, FD], F32, tag="in", bufs=8)
                nc.sync.dma_start(
                    out=t.rearrange("p (t w) -> p t w", t=NBLK),
                    in_=ins[key][b, c].rearrange("(t p) w -> p t w", p=128))
                img[f"{nm}{c}"] = t

        # map -> (product spec, is_square)
        maps = [
            ("num_r", [("pr0", "tr0", 0), ("pi0", "ti0", 0),
                       ("pr1", "tr1", 0), ("pi1", "ti1", 0)], False),
            ("num_i", [("pi0", "tr0", 0), ("pr0", "ti0", 1),
                       ("pi1", "tr1", 0), ("pr1", "ti1", 1)], False),
            ("den_p", [("pr0", None, 0), ("pi0", None, 0),
                       ("pr1", None, 0), ("pi1", None, 0)], True),
            ("den_t", [("tr0", None, 0), ("ti0", None, 0),
                       ("tr1", None, 0), ("ti1", None, 0)], True),
        ]
        GB[b] = {}
        for mname, prods, is_sq in maps:
            # ---- products (DVE muls / ACT squares) -----------------------
            ptiles = []
            for (a, bb, neg) in prods:
                pt = pool.tile([128, FD], F32R, tag="prod", bufs=8)
                if is_sq:
                    nc.scalar.activation(out=pt, in_=img[a], func=AF.Square)
                else:
                    nc.vector.tensor_mul(pt, img[a], img[bb])
                ptiles.append((pt, neg))

            # ---- H-box + channel sum on PE (banded matmuls) --------------
            psum = psum_pool.tile([128, FD], F32, tag="ps", bufs=2)
            for t in range(NBLK):
                mm = []
                for (pt, neg) in ptiles:
                    mm.append((band_main[neg], pt[:, t * W:(t + 1) * W]))
                if t < NBLK - 1:
                    for (pt, neg) in ptiles:
                        mm.append((band_bnd[neg],
                                   pt[0:4, (t + 1) * W:(t + 2) * W]))
                outband = psum[:, t * W:(t + 1) * W]
                for i, (lhsT, rhs) in enumerate(mm):
                    nc.tensor.matmul(outband, lhsT, rhs,
                                     start=(i == 0), stop=(i == len(mm) - 1))

            # Defer the W-box by one map: the next map's DVE products are
            # emitted ahead of this scan in the DVE FIFO, so DVE doesn't
            # stall waiting for this map's matmuls.
            pending.append((b, mname, psum))
            if len(pending) > 1:
                _flush_wbox(pending.pop(0))

    while pending:
        _flush_wbox(pending.pop(0))

    # ---- finals for all batches: abs_c = exp(0.5*(ln A - ln B)) ---------
    for b in range(BLOC):
        G = GB[b]
        t1 = pool.tile([128, GW], BF16, tag="f", bufs=6)
        nc.scalar.activation(out=t1, in_=G["num_r"], func=AF.Square)
        t2 = pool.tile([128, GW], BF16, tag="f", bufs=6)
        nc.scalar.activation(out=t2, in_=G["num_i"], func=AF.Square)
        a_t = pool.tile([128, GW], BF16, tag="f", bufs=6)
        nc.vector.tensor_add(a_t, t1, t2)
        b_t = pool.tile([128, GW], BF16, tag="f", bufs=6)
        nc.vector.tensor_mul(b_t, G["den_p"], G["den_t"])
        ln_a = pool.tile([128, GW], BF16, tag="f", bufs=6)
        nc.scalar.activation(out=ln_a, in_=a_t, func=AF.Ln)
        ln_b = pool.tile([128, GW], BF16, tag="f", bufs=6)
        nc.scalar.activation(out=ln_b, in_=b_t, func=AF.Ln)
        d_t = pool.tile([128, GW], BF16, tag="f", bufs=6)
        nc.vector.tensor_sub(d_t, ln_a, ln_b)

        sink = pool.tile([128, FULL], BF16, tag="sink", bufs=2)
        nc.scalar.activation(out=sink, in_=d_t[:, 0:FULL], func=AF.Exp,
                             scale=0.5, accum_out=acc[:, 2 * b:2 * b + 1])
        nc.scalar.activation(out=sink[0:TAIL_ROWS, 0:WP],
                             in_=d_t[0:TAIL_ROWS, FULL:GW], func=AF.Exp,
                             scale=0.5,
                             accum_out=acc[0:TAIL_ROWS, 2 * b + 1:2 * b + 2])

    nc.sync.dma_start(out=out_ap, in_=acc)


_NC_CACHE = None


def _get_nc():
    global _NC_CACHE
    if _NC_CACHE is None:
        _NC_CACHE = _build_nc()
    return _NC_CACHE


def _run(inputs: dict, trace: bool = False, **kw):
    nc = _get_nc()
    bands = _make_bands()
    full = {k: np.ascontiguousarray(np.asarray(inputs[k]), dtype=np.float32)
            for k in ("pred_real", "pred_imag", "tgt_real", "tgt_imag")}
    in_maps = []
    for i in range(NCORES):
        sl = slice(i * BLOC, (i + 1) * BLOC)
        m = {k: np.ascontiguousarray(v[sl]) for k, v in full.items()}
        m["bands"] = bands
        in_maps.append(m)
    res = run_bass_kernel_spmd(nc, in_maps, core_ids=list(range(NCORES)),
                               trace=trace, **kw)
    total = 0.0
    for r in res.results:
        total += r["partials"].astype(np.float64).sum()
    coh = total / (B * HP * WP)
    out = np.float32(1.0 - coh)
    return np.asarray(out, dtype=np.float32), res


def kernel(pred_real, pred_imag, tgt_real, tgt_imag, filter_size):
    assert int(filter_size) == KF, f"filter_size {filter_size} != {KF}"
    out, _ = _run(dict(pred_real=pred_real, pred_imag=pred_imag,
                       tgt_real=tgt_real, tgt_imag=tgt_imag))
    return out
